# revision 12
# baseline (speedup 1.0000x reference)
"""Converged Toeplitz inhibition kernel for TRN2 (8 NeuronCores, SPMD).

out[n, c, h, w] = sum_k act[n, k, h, w] * Winv[k, c]
where Winv = inv(I - circulant(pad_roll(inhibition_filter, C)))  [C x C]

Strategy (per sharding hint): the tiny C x C inverse is computed on the host
and replicated to every core; activations are sharded along batch N (32 -> 4
per core). Each core runs a dense [K=256] x [M=256] x [N_free=4*4096] matmul.

The kernel is HBM-bandwidth-bound (~358 GB/s per NeuronCore), so all wire
traffic is fp16: activations are cast to fp16 on the host, the matmul runs
fp16 x fp16 -> fp32 PSUM, and the output is written back as fp16 and upcast
to fp32 on the host. This halves HBM traffic vs fp32 (16.8 MB/core total)
for a ~47 us DMA floor; fp16's 11-bit mantissa keeps rel err ~1e-3, far
under the 2e-2 gate (the old fp32r path also had an 11-bit mantissa).

  - weights held in SBUF as four 128x128 views of one [128, 512] tile
  - activations DMA'd in as [128, 2048] fp16 chunks (0.5 MB) on the SP
    HWDGE ring
  - PE matmul fp16, free dim 512, PSUM fp32
  - PSUM -> SBUF evacuation (with fp32->fp16 cast) alternating
    ScalarE / VectorE
  - output DMA'd out as [128, 2048] fp16 chunks on the ACT HWDGE ring, so
    read and write streams overlap
"""

import numpy as np

import concourse.bass as bass
import concourse.bacc as bacc
import concourse.mybir as mybir
import concourse.tile as tile
from concourse.bass_utils import run_bass_kernel_spmd

N, C, H, W = 32, 256, 64, 64
HW = H * W  # 4096
NCORES = 8
NB = N // NCORES  # batches per core
P = 128  # partitions
FD = 512  # matmul free dim (one fp32 PSUM bank)

MM_DT = mybir.dt.float16


def _build_w(inhibition_filter: np.ndarray) -> np.ndarray:
    """Replicates reference._pad_roll + _circulant + inv(I - tpl) in numpy."""
    filt = np.asarray(inhibition_filter, dtype=np.float32)
    scope = filt.shape[0]
    pad_left = (C - scope) // 2
    padded = np.zeros(C, np.float32)
    padded[pad_left : pad_left + scope] = filt
    kernel = np.roll(padded, C // 2 + 1)
    idx = (np.arange(C)[None, :] - np.arange(C)[:, None]) % C
    tpl = kernel[idx].astype(np.float64)
    w = np.linalg.inv(np.eye(C, dtype=np.float64) - tpl)
    return np.ascontiguousarray(w.astype(np.float32))


def _body(tc: tile.TileContext, out, act, w):
    # In-DMAs ride the SP HWDGE ring (nc.sync), out-DMAs the ACT ring
    # (nc.scalar) so input and output streams don't serialize on one FIFO
    # ring.
    nc = tc.nc
    CH = 2048  # chunk width
    NCH = HW // CH  # 2 chunks per batch
    JPC = CH // FD  # 4 matmul free-dim slices per chunk
    with (
        tc.tile_pool(name="wpool", bufs=1) as wpool,
        tc.tile_pool(name="apool", bufs=4) as apool,
        tc.tile_pool(name="opool", bufs=3) as opool,
        tc.tile_pool(name="psum", bufs=1, space="PSUM") as pspool,
    ):
        # Weights arrive host-packed as [128, 512]: the four 128x128 tiles
        # (k-major, then m) side by side, so one DMA loads them all. It
        # rides the gpsimd SWDGE queue (a third DMA path, otherwise idle)
        # so neither HWDGE ring is delayed and it lands ~8 us in, before
        # the first activation chunk.
        wtile = wpool.tile([P, 4 * P], MM_DT, tag="w", name="wtile")
        nc.gpsimd.dma_start(out=wtile[:], in_=w[:, :])
        wt = [
            [wtile[:, (2 * k + m) * P : (2 * k + m + 1) * P] for m in range(2)]
            for k in range(2)
        ]

        # PE warmup: the HAM throttle starts the PE at half rate and needs
        # ~4 us of sustained matmuls to reach full rate. Burn the dead time
        # between weight arrival and first activation chunk on throwaway
        # matmuls over the weight tile itself (no extra SBUF, no
        # uninitialized reads); results land in PSUM and are discarded.
        pw = pspool.tile([P, CH], mybir.dt.float32, tag="psA", name="pw")
        for i in range(12):
            nc.tensor.matmul(
                pw[:, (i % JPC) * FD : (i % JPC + 1) * FD],
                lhsT=wtile[:, 0:P],
                rhs=wtile[:],
                start=True,
                stop=True,
            )

        for n in range(NB):
            last = n == NB - 1
            # The last batch drains at 1024-wide tiles: its output is the
            # only thing left on the wire, so finer quanta shorten the
            # serial matmul->copy->DMA pipeline at the end of the run.
            CHn = 1024 if last else CH
            NCHn = HW // CHn
            JPCn = CHn // FD
            ap = "b" if last else "a"
            a = {}
            for c in range(NCHn):
                for k in range(2):
                    a[k, c] = apool.tile(
                        [P, CHn],
                        MM_DT,
                        tag=f"{ap}{k}{c}",
                        name=f"{ap}{k}{c}",
                        bufs=1 if last else 4,
                    )
                    nc.sync.dma_start(
                        out=a[k, c][:],
                        in_=act[n, k * P : (k + 1) * P, c * CHn : (c + 1) * CHn],
                    )
            for c in range(NCHn):
                for m in range(2):
                    o = opool.tile(
                        [P, CHn],
                        MM_DT,
                        tag=f"{ap}o{m}{c}",
                        name=f"{ap}o{m}{c}",
                        bufs=1 if last else (3 if c == 0 else 2),
                    )
                    # One multi-bank PSUM tile per (c, m); matmuls fill its
                    # 512-wide slices (each slice = one bank = one
                    # accumulation group), k-outer so consecutive matmuls
                    # share stationary weights.
                    # PSUM tiles are always [P, CH] (4 banks, 2 tags = all 8
                    # banks); the last batch's 1024-wide tiles just use the
                    # first half.
                    ps = pspool.tile(
                        [P, CH], mybir.dt.float32, tag=f"ps{'AB'[m]}", name="ps"
                    )
                    for k in range(2):
                        for jj in range(JPCn):
                            nc.tensor.matmul(
                                ps[:, jj * FD : (jj + 1) * FD],
                                lhsT=wt[k][m],
                                rhs=a[k, c][:, jj * FD : (jj + 1) * FD],
                                start=(k == 0),
                                stop=(k == 1),
                            )
                    # Evacuate the tile as two concurrent half-copies, one
                    # per engine, so evacuation (~1 us) outruns the fabric
                    # and neither engine pays per-512-slice instruction
                    # overhead 4x.
                    HH = CHn // 2
                    nc.scalar.copy(o[:, 0:HH], ps[:, 0:HH])
                    nc.vector.tensor_copy(o[:, HH:CHn], ps[:, HH:CHn])
                    # Out-DMAs ride the scalar HWDGE ring (fast trigger);
                    # the last batch's ride the sync ring, which has
                    # finished all input issue by then, keeping both copy
                    # engines free for the drain.
                    dma_eng = nc.sync if last else nc.scalar
                    dma_eng.dma_start(
                        out=out[n, m * P : (m + 1) * P, c * CHn : (c + 1) * CHn],
                        in_=o[:],
                    )


_NC_CACHE = None


def _get_nc():
    global _NC_CACHE
    if _NC_CACHE is None:
        nc = bacc.Bacc(
            "TRN2", debug=False, enable_asserts=False, enable_partition_id=False
        )
        act = nc.dram_tensor("act", [NB, C, HW], MM_DT, kind="ExternalInput").ap()
        w = nc.dram_tensor("w", [P, 4 * P], MM_DT, kind="ExternalInput").ap()
        out = nc.dram_tensor("out", [NB, C, HW], MM_DT, kind="ExternalOutput").ap()
        with tile.TileContext(nc) as tc:
            _body(tc, out, act, w)
        nc.compile()
        _NC_CACHE = nc
    return _NC_CACHE


def _run(activations: np.ndarray, w: np.ndarray, trace: bool = False):
    acts = (
        np.ascontiguousarray(activations, dtype=np.float32)
        .astype(np.float16)
        .reshape(NCORES, NB, C, HW)
    )
    # Pack w [256, 256] into [128, 1024]: four 128x128 tiles (k-major, then
    # m) side by side, matching the single weight DMA + wt views on-device.
    w16 = w.astype(np.float16)
    wp = np.empty((P, 4 * P), np.float16)
    for k in range(2):
        for m in range(2):
            wp[:, (2 * k + m) * P : (2 * k + m + 1) * P] = w16[
                k * P : (k + 1) * P, m * P : (m + 1) * P
            ]
    wp = np.ascontiguousarray(wp)
    in_maps = [{"act": acts[i], "w": wp} for i in range(NCORES)]
    nc = _get_nc()
    res = run_bass_kernel_spmd(nc, in_maps, list(range(NCORES)), trace=trace)
    out = np.concatenate([res.results[i]["out"] for i in range(NCORES)], axis=0)
    return out.astype(np.float32).reshape(N, C, H, W), res


def kernel(activations: np.ndarray, inhibition_filter: np.ndarray) -> np.ndarray:
    w = _build_w(inhibition_filter)
    out, _ = _run(activations, w, trace=False)
    return out


# revision 13
# speedup vs baseline: 1.1117x; 1.1117x over previous
"""Converged Toeplitz inhibition kernel for TRN2 (8 NeuronCores, SPMD).

out[n, c, h, w] = sum_k act[n, k, h, w] * Winv[k, c]
where Winv = inv(I - circulant(pad_roll(inhibition_filter, C)))  [C x C]

Strategy (per sharding hint): the tiny C x C inverse is computed on the host
and replicated to every core; activations are sharded along batch N (32 -> 4
per core). Each core runs a dense [K=256] x [M=256] x [N_free=4*4096] matmul.

The kernel is HBM-bandwidth-bound (~358 GB/s per NeuronCore), so all wire
traffic is fp16: activations are cast to fp16 on the host, the matmul runs
fp16 x fp16 -> fp32 PSUM, and the output is written back as fp16 and upcast
to fp32 on the host. This halves HBM traffic vs fp32 (16.8 MB/core total)
for a ~47 us DMA floor; fp16's 11-bit mantissa keeps rel err ~1e-3, far
under the 2e-2 gate (the old fp32r path also had an 11-bit mantissa).

  - weights held in SBUF as four 128x128 views of one [128, 512] tile
  - activations DMA'd in as [128, 2048] fp16 chunks (0.5 MB) on the SP
    HWDGE ring
  - PE matmul fp16, free dim 512, PSUM fp32
  - PSUM -> SBUF evacuation (with fp32->fp16 cast) alternating
    ScalarE / VectorE
  - output DMA'd out as [128, 2048] fp16 chunks on the ACT HWDGE ring, so
    read and write streams overlap
"""

import numpy as np

import concourse.bass as bass
import concourse.bacc as bacc
import concourse.mybir as mybir
import concourse.tile as tile
from concourse.bass_utils import run_bass_kernel_spmd

N, C, H, W = 32, 256, 64, 64
HW = H * W  # 4096
NCORES = 8
NB = N // NCORES  # batches per core
P = 128  # partitions
FD = 512  # matmul free dim (one fp32 PSUM bank)

MM_DT = mybir.dt.float16


def _build_w(inhibition_filter: np.ndarray) -> np.ndarray:
    """Replicates reference._pad_roll + _circulant + inv(I - tpl) in numpy."""
    filt = np.asarray(inhibition_filter, dtype=np.float32)
    scope = filt.shape[0]
    pad_left = (C - scope) // 2
    padded = np.zeros(C, np.float32)
    padded[pad_left : pad_left + scope] = filt
    kernel = np.roll(padded, C // 2 + 1)
    idx = (np.arange(C)[None, :] - np.arange(C)[:, None]) % C
    tpl = kernel[idx].astype(np.float64)
    w = np.linalg.inv(np.eye(C, dtype=np.float64) - tpl)
    return np.ascontiguousarray(w.astype(np.float32))


def _body(tc: tile.TileContext, out, act, w):
    # In-DMAs ride the SP HWDGE ring (nc.sync), out-DMAs the ACT ring
    # (nc.scalar) so input and output streams don't serialize on one FIFO
    # ring.
    nc = tc.nc
    CH = 2048  # chunk width
    NCH = HW // CH  # 2 chunks per batch
    JPC = CH // FD  # 4 matmul free-dim slices per chunk
    with (
        tc.tile_pool(name="wpool", bufs=1) as wpool,
        tc.tile_pool(name="apool", bufs=4) as apool,
        tc.tile_pool(name="opool", bufs=3) as opool,
        tc.tile_pool(name="psum", bufs=1, space="PSUM") as pspool,
    ):
        # Weights arrive host-packed as [128, 512]: the four 128x128 tiles
        # (k-major, then m) side by side, so one DMA loads them all. It
        # rides the gpsimd SWDGE queue (a third DMA path, otherwise idle)
        # so neither HWDGE ring is delayed and it lands ~8 us in, before
        # the first activation chunk.
        wtile = wpool.tile([P, 4 * P], MM_DT, tag="w", name="wtile")
        nc.gpsimd.dma_start(out=wtile[:], in_=w[:, :])
        wt = [
            [wtile[:, (2 * k + m) * P : (2 * k + m + 1) * P] for m in range(2)]
            for k in range(2)
        ]

        # PE warmup: the HAM throttle starts the PE at half rate and needs
        # ~4 us of sustained matmuls to reach full rate. Burn the dead time
        # between weight arrival and first activation chunk on throwaway
        # matmuls over the weight tile itself (no extra SBUF, no
        # uninitialized reads); results land in PSUM and are discarded.
        pw = pspool.tile([P, CH], mybir.dt.float32, tag="psA", name="pw")
        for i in range(12):
            nc.tensor.matmul(
                pw[:, (i % JPC) * FD : (i % JPC + 1) * FD],
                lhsT=wtile[:, 0:P],
                rhs=wtile[:],
                start=True,
                stop=True,
            )

        for n in range(NB):
            last = n == NB - 1
            # The last batch drains at 1024-wide tiles: its output is the
            # only thing left on the wire, so finer quanta shorten the
            # serial matmul->copy->DMA pipeline at the end of the run.
            CHn = 1024 if last else CH
            NCHn = HW // CHn
            JPCn = CHn // FD
            ap = "b" if last else "a"
            a = {}
            for c in range(NCHn):
                for k in range(2):
                    a[k, c] = apool.tile(
                        [P, CHn],
                        MM_DT,
                        tag=f"{ap}{k}{c}",
                        name=f"{ap}{k}{c}",
                        bufs=1 if last else 4,
                    )
                    nc.sync.dma_start(
                        out=a[k, c][:],
                        in_=act[n, k * P : (k + 1) * P, c * CHn : (c + 1) * CHn],
                    )
            for c in range(NCHn):
                for m in range(2):
                    o = opool.tile(
                        [P, CHn],
                        MM_DT,
                        tag=f"{ap}o{m}{c}",
                        name=f"{ap}o{m}{c}",
                        bufs=1 if last else (3 if c == 0 else 2),
                    )
                    # One multi-bank PSUM tile per (c, m); matmuls fill its
                    # 512-wide slices (each slice = one bank = one
                    # accumulation group), k-outer so consecutive matmuls
                    # share stationary weights.
                    # PSUM tiles are always [P, CH] (4 banks, 2 tags = all 8
                    # banks); the last batch's 1024-wide tiles just use the
                    # first half.
                    ps = pspool.tile(
                        [P, CH], mybir.dt.float32, tag=f"ps{'AB'[m]}", name="ps"
                    )
                    for k in range(2):
                        for jj in range(JPCn):
                            nc.tensor.matmul(
                                ps[:, jj * FD : (jj + 1) * FD],
                                lhsT=wt[k][m],
                                rhs=a[k, c][:, jj * FD : (jj + 1) * FD],
                                start=(k == 0),
                                stop=(k == 1),
                            )
                    # Evacuate the tile as two concurrent half-copies, one
                    # per engine, so evacuation (~1 us) outruns the fabric
                    # and neither engine pays per-512-slice instruction
                    # overhead 4x.
                    HH = CHn // 2
                    nc.scalar.copy(o[:, 0:HH], ps[:, 0:HH])
                    nc.vector.tensor_copy(o[:, HH:CHn], ps[:, HH:CHn])
                    # Out-DMA engine choice:
                    # - first chunk (n0,c0): scalar HWDGE — fastest trigger,
                    #   pulls the out-stream onset ~3 us earlier; the one-off
                    #   HOL cost on scalar's copy queue is tiny.
                    # - last batch: alternate sync/gpsimd so the drain's
                    #   per-DMA queue overhead overlaps across two queues
                    #   (sync has finished all input issue by then).
                    # - everything else: gpsimd SWDGE, keeping both copy
                    #   engines free.
                    if n == 0 and c == 0:
                        dma_eng = nc.scalar
                    elif last:
                        dma_eng = nc.sync if (c * 2 + m) % 2 == 0 else nc.gpsimd
                    else:
                        dma_eng = nc.gpsimd
                    dma_eng.dma_start(
                        out=out[n, m * P : (m + 1) * P, c * CHn : (c + 1) * CHn],
                        in_=o[:],
                    )


_NC_CACHE = None


def _get_nc():
    global _NC_CACHE
    if _NC_CACHE is None:
        nc = bacc.Bacc(
            "TRN2", debug=False, enable_asserts=False, enable_partition_id=False
        )
        act = nc.dram_tensor("act", [NB, C, HW], MM_DT, kind="ExternalInput").ap()
        w = nc.dram_tensor("w", [P, 4 * P], MM_DT, kind="ExternalInput").ap()
        out = nc.dram_tensor("out", [NB, C, HW], MM_DT, kind="ExternalOutput").ap()
        with tile.TileContext(nc) as tc:
            _body(tc, out, act, w)
        nc.compile()
        _NC_CACHE = nc
    return _NC_CACHE


def _run(activations: np.ndarray, w: np.ndarray, trace: bool = False):
    acts = (
        np.ascontiguousarray(activations, dtype=np.float32)
        .astype(np.float16)
        .reshape(NCORES, NB, C, HW)
    )
    # Pack w [256, 256] into [128, 1024]: four 128x128 tiles (k-major, then
    # m) side by side, matching the single weight DMA + wt views on-device.
    w16 = w.astype(np.float16)
    wp = np.empty((P, 4 * P), np.float16)
    for k in range(2):
        for m in range(2):
            wp[:, (2 * k + m) * P : (2 * k + m + 1) * P] = w16[
                k * P : (k + 1) * P, m * P : (m + 1) * P
            ]
    wp = np.ascontiguousarray(wp)
    in_maps = [{"act": acts[i], "w": wp} for i in range(NCORES)]
    nc = _get_nc()
    res = run_bass_kernel_spmd(nc, in_maps, list(range(NCORES)), trace=trace)
    out = np.concatenate([res.results[i]["out"] for i in range(NCORES)], axis=0)
    return out.astype(np.float32).reshape(N, C, H, W), res


def kernel(activations: np.ndarray, inhibition_filter: np.ndarray) -> np.ndarray:
    w = _build_w(inhibition_filter)
    out, _ = _run(activations, w, trace=False)
    return out


# revision 21
# speedup vs baseline: 1.1468x; 1.0316x over previous
"""Converged Toeplitz inhibition kernel for TRN2 (8 NeuronCores, SPMD).

out[n, c, h, w] = sum_k act[n, k, h, w] * Winv[k, c]
where Winv = inv(I - circulant(pad_roll(inhibition_filter, C)))  [C x C]

Strategy (per sharding hint): the tiny C x C inverse is computed on the host
and replicated to every core; activations are sharded along batch N (32 -> 4
per core). Each core runs a dense [K=256] x [M=256] x [N_free=4*4096] matmul.

The kernel is HBM-bandwidth-bound (~358 GB/s per NeuronCore), so all wire
traffic is fp16: activations are cast to fp16 on the host, the matmul runs
fp16 x fp16 -> fp32 PSUM, and the output is written back as fp16 and upcast
to fp32 on the host. This halves HBM traffic vs fp32 (16.8 MB/core total)
for a ~47 us DMA floor; fp16's 11-bit mantissa keeps rel err ~1e-3, far
under the 2e-2 gate (the old fp32r path also had an 11-bit mantissa).

  - weights held in SBUF as four 128x128 views of one [128, 512] tile
  - activations DMA'd in as [128, 2048] fp16 chunks (0.5 MB) on the SP
    HWDGE ring
  - PE matmul fp16, free dim 512, PSUM fp32
  - PSUM -> SBUF evacuation (with fp32->fp16 cast) alternating
    ScalarE / VectorE
  - output DMA'd out as [128, 2048] fp16 chunks on the ACT HWDGE ring, so
    read and write streams overlap
"""

import numpy as np

import concourse.bass as bass
import concourse.bacc as bacc
import concourse.mybir as mybir
import concourse.tile as tile
from concourse.bass_utils import run_bass_kernel_spmd

N, C, H, W = 32, 256, 64, 64
HW = H * W  # 4096
NCORES = 8
NB = N // NCORES  # batches per core
P = 128  # partitions
FD = 512  # matmul free dim (one fp32 PSUM bank)

MM_DT = mybir.dt.float16


def _build_w(inhibition_filter: np.ndarray) -> np.ndarray:
    """Replicates reference._pad_roll + _circulant + inv(I - tpl) in numpy."""
    filt = np.asarray(inhibition_filter, dtype=np.float32)
    scope = filt.shape[0]
    pad_left = (C - scope) // 2
    padded = np.zeros(C, np.float32)
    padded[pad_left : pad_left + scope] = filt
    kernel = np.roll(padded, C // 2 + 1)
    idx = (np.arange(C)[None, :] - np.arange(C)[:, None]) % C
    tpl = kernel[idx].astype(np.float64)
    w = np.linalg.inv(np.eye(C, dtype=np.float64) - tpl)
    return np.ascontiguousarray(w.astype(np.float32))


# Schedule knobs (A/B tested; defaults = best measured config).
#   drain_ch:   chunk width for the last batch (1024 = finer drain quanta)
#   first_out:  engine for batch-0 chunk-0 out-DMAs ("scalar" pulls the
#               out-stream onset earlier; "gpsimd" keeps scalar copy-only)
#   drain_eng:  out-DMA engine(s) for the last batch
CFG = {"drain_ch": 2048, "first_out": "gpsimd", "drain_eng": "sync"}


def _body(tc: tile.TileContext, out, act, w, cfg=None):
    # In-DMAs ride the SP HWDGE ring (nc.sync), out-DMAs the ACT ring
    # (nc.scalar) so input and output streams don't serialize on one FIFO
    # ring.
    cfg = dict(CFG, **(cfg or {}))
    nc = tc.nc
    CH = 2048  # chunk width
    NCH = HW // CH  # 2 chunks per batch
    JPC = CH // FD  # 4 matmul free-dim slices per chunk
    with (
        tc.tile_pool(name="wpool", bufs=1) as wpool,
        tc.tile_pool(name="apool", bufs=4) as apool,
        tc.tile_pool(name="opool", bufs=3) as opool,
        tc.tile_pool(name="psum", bufs=1, space="PSUM") as pspool,
    ):
        # Weights arrive host-packed as [128, 512]: the four 128x128 tiles
        # (k-major, then m) side by side, so one DMA loads them all. It
        # rides the gpsimd SWDGE queue (a third DMA path, otherwise idle)
        # so neither HWDGE ring is delayed and it lands ~8 us in, before
        # the first activation chunk.
        wtile = wpool.tile([P, 4 * P], MM_DT, tag="w", name="wtile")
        nc.gpsimd.dma_start(out=wtile[:], in_=w[:, :])
        wt = [
            [wtile[:, (2 * k + m) * P : (2 * k + m + 1) * P] for m in range(2)]
            for k in range(2)
        ]

        # PE warmup: the HAM throttle starts the PE at half rate and needs
        # ~4 us of sustained matmuls to reach full rate. Burn the dead time
        # between weight arrival and first activation chunk on throwaway
        # matmuls over the weight tile itself (no extra SBUF, no
        # uninitialized reads); results land in PSUM and are discarded.
        pw = pspool.tile([P, CH], mybir.dt.float32, tag="psA", name="pw")
        for i in range(12):
            nc.tensor.matmul(
                pw[:, (i % JPC) * FD : (i % JPC + 1) * FD],
                lhsT=wtile[:, 0:P],
                rhs=wtile[:],
                start=True,
                stop=True,
            )

        for n in range(NB):
            last = n == NB - 1
            # The last batch drains at finer tiles: its output is the only
            # thing left on the wire, so finer quanta shorten the serial
            # matmul->copy->DMA pipeline at the end of the run.
            CHn = cfg["drain_ch"] if last else CH
            NCHn = HW // CHn
            JPCn = CHn // FD
            ap = "b" if last else "a"
            a = {}
            for c in range(NCHn):
                for k in range(2):
                    a[k, c] = apool.tile(
                        [P, CHn],
                        MM_DT,
                        tag=f"{ap}{k}{c}",
                        name=f"{ap}{k}{c}",
                        bufs=1 if last else 4,
                    )
                    nc.sync.dma_start(
                        out=a[k, c][:],
                        in_=act[n, k * P : (k + 1) * P, c * CHn : (c + 1) * CHn],
                    )
            for c in range(NCHn):
                for m in range(2):
                    o = opool.tile(
                        [P, CHn],
                        MM_DT,
                        tag=f"{ap}o{m}{c}",
                        name=f"{ap}o{m}{c}",
                        bufs=1 if last else (3 if c == 0 else 2),
                    )
                    # One multi-bank PSUM tile per (c, m); matmuls fill its
                    # 512-wide slices (each slice = one bank = one
                    # accumulation group), k-outer so consecutive matmuls
                    # share stationary weights.
                    # PSUM tiles are always [P, CH] (4 banks, 2 tags = all 8
                    # banks); the last batch's 1024-wide tiles just use the
                    # first half.
                    ps = pspool.tile(
                        [P, CH], mybir.dt.float32, tag=f"ps{'AB'[m]}", name="ps"
                    )
                    for k in range(2):
                        for jj in range(JPCn):
                            nc.tensor.matmul(
                                ps[:, jj * FD : (jj + 1) * FD],
                                lhsT=wt[k][m],
                                rhs=a[k, c][:, jj * FD : (jj + 1) * FD],
                                start=(k == 0),
                                stop=(k == 1),
                            )
                    # Evacuate the tile as two concurrent half-copies, one
                    # per engine, so evacuation (~1 us) outruns the fabric
                    # and neither engine pays per-512-slice instruction
                    # overhead 4x.
                    HH = CHn // 2
                    nc.scalar.copy(o[:, 0:HH], ps[:, 0:HH])
                    nc.vector.tensor_copy(o[:, HH:CHn], ps[:, HH:CHn])
                    # Out-DMA engine choice:
                    # - first chunk (n0,c0): scalar HWDGE — fastest trigger,
                    #   pulls the out-stream onset ~3 us earlier; the one-off
                    #   HOL cost on scalar's copy queue is tiny.
                    # - last batch: alternate sync/gpsimd so the drain's
                    #   per-DMA queue overhead overlaps across two queues
                    #   (sync has finished all input issue by then).
                    # - everything else: gpsimd SWDGE, keeping both copy
                    #   engines free.
                    if n == 0 and c == 0:
                        dma_eng = getattr(nc, cfg["first_out"])
                    elif last:
                        de = cfg["drain_eng"]
                        if de == "alt":
                            de = "sync" if (c * 2 + m) % 2 == 0 else "gpsimd"
                        dma_eng = getattr(nc, de)
                    else:
                        dma_eng = nc.gpsimd
                    dma_eng.dma_start(
                        out=out[n, m * P : (m + 1) * P, c * CHn : (c + 1) * CHn],
                        in_=o[:],
                    )


_NC_CACHE = {}


def _get_nc(cfg=None):
    key = tuple(sorted(dict(CFG, **(cfg or {})).items()))
    if key not in _NC_CACHE:
        nc = bacc.Bacc(
            "TRN2", debug=False, enable_asserts=False, enable_partition_id=False
        )
        act = nc.dram_tensor("act", [NB, C, HW], MM_DT, kind="ExternalInput").ap()
        w = nc.dram_tensor("w", [P, 4 * P], MM_DT, kind="ExternalInput").ap()
        out = nc.dram_tensor("out", [NB, C, HW], MM_DT, kind="ExternalOutput").ap()
        with tile.TileContext(nc) as tc:
            _body(tc, out, act, w, cfg)
        nc.compile()
        _NC_CACHE[key] = nc
    return _NC_CACHE[key]


def _run(activations: np.ndarray, w: np.ndarray, trace: bool = False, cfg=None):
    acts = (
        np.ascontiguousarray(activations, dtype=np.float32)
        .astype(np.float16)
        .reshape(NCORES, NB, C, HW)
    )
    # Pack w [256, 256] into [128, 1024]: four 128x128 tiles (k-major, then
    # m) side by side, matching the single weight DMA + wt views on-device.
    w16 = w.astype(np.float16)
    wp = np.empty((P, 4 * P), np.float16)
    for k in range(2):
        for m in range(2):
            wp[:, (2 * k + m) * P : (2 * k + m + 1) * P] = w16[
                k * P : (k + 1) * P, m * P : (m + 1) * P
            ]
    wp = np.ascontiguousarray(wp)
    in_maps = [{"act": acts[i], "w": wp} for i in range(NCORES)]
    nc = _get_nc(cfg)
    res = run_bass_kernel_spmd(nc, in_maps, list(range(NCORES)), trace=trace)
    out = np.concatenate([res.results[i]["out"] for i in range(NCORES)], axis=0)
    return out.astype(np.float32).reshape(N, C, H, W), res


def kernel(activations: np.ndarray, inhibition_filter: np.ndarray) -> np.ndarray:
    w = _build_w(inhibition_filter)
    out, _ = _run(activations, w, trace=False)
    return out


# revision 24
# speedup vs baseline: 1.2472x; 1.0875x over previous
"""Converged Toeplitz inhibition kernel for TRN2 (8 NeuronCores, SPMD).

out[n, c, h, w] = sum_k act[n, k, h, w] * Winv[k, c]
where Winv = inv(I - circulant(pad_roll(inhibition_filter, C)))  [C x C]

Strategy (per sharding hint): the tiny C x C inverse is computed on the host
and replicated to every core; activations are sharded along batch N (32 -> 4
per core). Each core runs a dense [K=256] x [M=256] x [N_free=4*4096] matmul.

The kernel is HBM-bandwidth-bound (~358 GB/s per NeuronCore), so all wire
traffic is fp16: activations are cast to fp16 on the host, the matmul runs
fp16 x fp16 -> fp32 PSUM, and the output is written back as fp16 and upcast
to fp32 on the host. This halves HBM traffic vs fp32 (16.8 MB/core total)
for a ~47 us DMA floor; fp16's 11-bit mantissa keeps rel err ~1e-3, far
under the 2e-2 gate (the old fp32r path also had an 11-bit mantissa).

  - weights held in SBUF as four 128x128 views of one [128, 512] tile
  - activations DMA'd in as [128, 2048] fp16 chunks (0.5 MB) on the SP
    HWDGE ring
  - PE matmul fp16, free dim 512, PSUM fp32
  - PSUM -> SBUF evacuation (with fp32->fp16 cast) alternating
    ScalarE / VectorE
  - output DMA'd out as [128, 2048] fp16 chunks on the ACT HWDGE ring, so
    read and write streams overlap
"""

import numpy as np

import concourse.bass as bass
import concourse.bacc as bacc
import concourse.mybir as mybir
import concourse.tile as tile
from concourse.bass_utils import run_bass_kernel_spmd

N, C, H, W = 32, 256, 64, 64
HW = H * W  # 4096
NCORES = 8
NB = N // NCORES  # batches per core
P = 128  # partitions
FD = 512  # matmul free dim (one fp32 PSUM bank)

MM_DT = mybir.dt.float16


def _build_w(inhibition_filter: np.ndarray) -> np.ndarray:
    """Replicates reference._pad_roll + _circulant + inv(I - tpl) in numpy."""
    filt = np.asarray(inhibition_filter, dtype=np.float32)
    scope = filt.shape[0]
    pad_left = (C - scope) // 2
    padded = np.zeros(C, np.float32)
    padded[pad_left : pad_left + scope] = filt
    kernel = np.roll(padded, C // 2 + 1)
    idx = (np.arange(C)[None, :] - np.arange(C)[:, None]) % C
    tpl = kernel[idx].astype(np.float64)
    w = np.linalg.inv(np.eye(C, dtype=np.float64) - tpl)
    return np.ascontiguousarray(w.astype(np.float32))


# Schedule knobs (A/B tested; defaults = best measured config).
#   drain_ch:   chunk width for the last batch (1024 = finer drain quanta)
#   first_out:  engine for batch-0 chunk-0 out-DMAs ("scalar" pulls the
#               out-stream onset earlier; "gpsimd" keeps scalar copy-only)
#   drain_eng:  out-DMA engine(s) for the last batch
CFG = {"drain_ch": 2048, "first_out": "gpsimd", "drain_eng": "sync"}


def _body(tc: tile.TileContext, out, act, w, cfg=None):
    # In-DMAs ride the SP HWDGE ring (nc.sync), out-DMAs the ACT ring
    # (nc.scalar) so input and output streams don't serialize on one FIFO
    # ring.
    cfg = dict(CFG, **(cfg or {}))
    nc = tc.nc
    CH = 2048  # chunk width
    NCH = HW // CH  # 2 chunks per batch
    JPC = CH // FD  # 4 matmul free-dim slices per chunk
    with (
        tc.tile_pool(name="wpool", bufs=1) as wpool,
        tc.tile_pool(name="apool", bufs=4) as apool,
        tc.tile_pool(name="opool", bufs=3) as opool,
        tc.tile_pool(name="psum", bufs=2, space="PSUM") as pspool,
    ):
        # Weights arrive host-packed as [128, 512]: the four 128x128 tiles
        # (k-major, then m) side by side, so one DMA loads them all. It
        # rides the gpsimd SWDGE queue (a third DMA path, otherwise idle)
        # so neither HWDGE ring is delayed and it lands ~8 us in, before
        # the first activation chunk.
        wtile = wpool.tile([P, 4 * P], MM_DT, tag="w", name="wtile")
        nc.gpsimd.dma_start(out=wtile[:], in_=w[:, :])
        wt = [
            [wtile[:, (2 * k + m) * P : (2 * k + m + 1) * P] for m in range(2)]
            for k in range(2)
        ]

        # PE warmup: the HAM throttle starts the PE at half rate and needs
        # ~4 us of sustained matmuls to reach full rate. Burn the dead time
        # between weight arrival and first activation chunk on throwaway
        # matmuls over the weight tile itself (no extra SBUF, no
        # uninitialized reads); results land in PSUM and are discarded.
        for i in range(12):
            pw = pspool.tile(
                [P, CH // 2], mybir.dt.float32, tag=f"ps{'AB'[i % 2]}", name="pw"
            )
            nc.tensor.matmul(
                pw[:, 0:FD], lhsT=wtile[:, 0:P], rhs=wtile[:], start=True, stop=True
            )

        for n in range(NB):
            last = n == NB - 1
            # The last batch drains at finer tiles: its output is the only
            # thing left on the wire, so finer quanta shorten the serial
            # matmul->copy->DMA pipeline at the end of the run.
            CHn = cfg["drain_ch"] if last else CH
            NCHn = HW // CHn
            JPCn = CHn // FD
            ap = "b" if last else "a"
            a = {}
            for c in range(NCHn):
                for k in range(2):
                    a[k, c] = apool.tile(
                        [P, CHn],
                        MM_DT,
                        tag=f"{ap}{k}{c}",
                        name=f"{ap}{k}{c}",
                        bufs=1 if last else 4,
                    )
                    nc.sync.dma_start(
                        out=a[k, c][:],
                        in_=act[n, k * P : (k + 1) * P, c * CHn : (c + 1) * CHn],
                    )
            for c in range(NCHn):
                for m in range(2):
                    o = opool.tile(
                        [P, CHn],
                        MM_DT,
                        tag=f"{ap}o{m}{c}",
                        name=f"{ap}o{m}{c}",
                        bufs=1 if last else (3 if c == 0 else 2),
                    )
                    # Two 2-bank PSUM tiles per (c, m) — a 4-deep rotation
                    # across the 8 banks (vs a marginal 2-deep ping-pong of
                    # 4-bank tiles, which stalled the PE ~0.7 us per chunk
                    # waiting on copies). Each half finishes its k
                    # accumulation before the other half starts, so its
                    # copy overlaps the PE filling the second half.
                    HH = CHn // 2
                    ps = [
                        pspool.tile(
                            [P, CH // 2], mybir.dt.float32, tag=f"ps{'AB'[h]}", name="ps"
                        )
                        for h in range(2)
                    ]
                    for h in range(2):
                        for k in range(2):
                            for jj in range(HH // FD):
                                col = h * HH + jj * FD
                                nc.tensor.matmul(
                                    ps[h][:, jj * FD : (jj + 1) * FD],
                                    lhsT=wt[k][m],
                                    rhs=a[k, c][:, col : col + FD],
                                    start=(k == 0),
                                    stop=(k == 1),
                                )
                    # Evacuate as two concurrent half-copies, one per engine.
                    nc.scalar.copy(o[:, 0:HH], ps[0][:, 0:HH])
                    nc.vector.tensor_copy(o[:, HH:CHn], ps[1][:, 0:HH])
                    # Out-DMA engine choice:
                    # - first chunk (n0,c0): scalar HWDGE — fastest trigger,
                    #   pulls the out-stream onset ~3 us earlier; the one-off
                    #   HOL cost on scalar's copy queue is tiny.
                    # - last batch: alternate sync/gpsimd so the drain's
                    #   per-DMA queue overhead overlaps across two queues
                    #   (sync has finished all input issue by then).
                    # - everything else: gpsimd SWDGE, keeping both copy
                    #   engines free.
                    if n == 0 and c == 0:
                        dma_eng = getattr(nc, cfg["first_out"])
                    elif last:
                        de = cfg["drain_eng"]
                        if de == "alt":
                            de = "sync" if (c * 2 + m) % 2 == 0 else "gpsimd"
                        dma_eng = getattr(nc, de)
                    else:
                        dma_eng = nc.gpsimd
                    dma_eng.dma_start(
                        out=out[n, m * P : (m + 1) * P, c * CHn : (c + 1) * CHn],
                        in_=o[:],
                    )


_NC_CACHE = {}


def _get_nc(cfg=None):
    key = tuple(sorted(dict(CFG, **(cfg or {})).items()))
    if key not in _NC_CACHE:
        nc = bacc.Bacc(
            "TRN2", debug=False, enable_asserts=False, enable_partition_id=False
        )
        act = nc.dram_tensor("act", [NB, C, HW], MM_DT, kind="ExternalInput").ap()
        w = nc.dram_tensor("w", [P, 4 * P], MM_DT, kind="ExternalInput").ap()
        out = nc.dram_tensor("out", [NB, C, HW], MM_DT, kind="ExternalOutput").ap()
        with tile.TileContext(nc) as tc:
            _body(tc, out, act, w, cfg)
        nc.compile()
        _NC_CACHE[key] = nc
    return _NC_CACHE[key]


def _run(activations: np.ndarray, w: np.ndarray, trace: bool = False, cfg=None):
    acts = (
        np.ascontiguousarray(activations, dtype=np.float32)
        .astype(np.float16)
        .reshape(NCORES, NB, C, HW)
    )
    # Pack w [256, 256] into [128, 1024]: four 128x128 tiles (k-major, then
    # m) side by side, matching the single weight DMA + wt views on-device.
    w16 = w.astype(np.float16)
    wp = np.empty((P, 4 * P), np.float16)
    for k in range(2):
        for m in range(2):
            wp[:, (2 * k + m) * P : (2 * k + m + 1) * P] = w16[
                k * P : (k + 1) * P, m * P : (m + 1) * P
            ]
    wp = np.ascontiguousarray(wp)
    in_maps = [{"act": acts[i], "w": wp} for i in range(NCORES)]
    nc = _get_nc(cfg)
    res = run_bass_kernel_spmd(nc, in_maps, list(range(NCORES)), trace=trace)
    out = np.concatenate([res.results[i]["out"] for i in range(NCORES)], axis=0)
    return out.astype(np.float32).reshape(N, C, H, W), res


def kernel(activations: np.ndarray, inhibition_filter: np.ndarray) -> np.ndarray:
    w = _build_w(inhibition_filter)
    out, _ = _run(activations, w, trace=False)
    return out
